# revision 56
# baseline (speedup 1.0000x reference)
"""Trainium2 Bass kernel: dense transformer block (B=2, S=2048, D=1024, H=16, DFF=4096).

Strategy: sequence-parallel across 8 NeuronCores (2 batches x 4 cores). Each core
owns 4 query-chunks of 128 tokens, interleaved {j, 7-j, 8+j, 15-j} so causal
attention work is balanced; per-core causal depth is padded to fixed slot budgets
(16, 12, 8, 4) with host-supplied 0/1 masks making the padding exact.

Optimizations vs the original baseline:
- bf16 matmul operands everywhere (weights, activations, probs); the residual
  path stays f32 (rel err ~1.5e-3 vs the 2e-2 gate).
- K^T and V (+softmax-denominator ones columns) are packed into merged
  AllGather buffers laid out exactly as attention consumes them, split into
  two collectives: the deep key-blocks (slots 3,2) gather right after their
  projections finish so attention sub-pass A overlaps the second gather.
- Attention: per head, one PSUM accumulation per sub-pass. Same-width score
  blocks share one 2-bank PSUM tile (pairs in sub-pass A, quads in B), so
  exp and causal-mask run as single wide instructions — the scalar engine
  pays ~185 ns of SBUF/PSUM access latency per instruction. Softmax
  reciprocals are computed in place on partition 64 (ones-row) and broadcast
  to 128 partitions by two rank-1 matmuls against a selector row — no
  cross-partition DMA in the normalize chain.
- DMAs are merged into whole-tile transfers (HWDGE descriptor generation is
  ~630 ns per DMA, serialized); Wo@Wp, part of W1, and the f32 residual
  prefetch into SBUF during the DMA-idle attention window, queued behind the
  gathered-KV loads so they cannot delay them.
- Engine balance: exp exclusively on ACT; bias-adds/masks/copies on DVE;
  gelu on ACT (idle during the FFN).
Wo@Wp is fused on the host; 1/sqrt(64) is folded into Wq (exact power of two).
"""
import numpy as np

B, S, D, H, W, DFF = 2, 2048, 1024, 16, 64, 4096
N_CORES = 8
TOK = 512            # tokens per core
NKB = 16             # key blocks (of 128 tokens) per batch
KVC = 8 * 128 + 8 * 130   # kv row: K block cols + V(+ones) block cols = 2064
N_W1_PRE = 8         # W1 tiles preloaded to SBUF during attention

_CACHE = {}


def _chunk_rank_slot(c):
    """Global 128-token chunk c (0..15) -> (group-rank, slot). Rank j owns
    chunks {j, 7-j, 8+j, 15-j}, stored in slot order sorted by causal depth
    descending: slots = [15-j, 8+j, 7-j, j]."""
    if c < 4:
        return c, 3
    if c < 8:
        return 7 - c, 2
    if c < 12:
        return c - 8, 1
    return 15 - c, 0


def _rank_chunks(j):
    """Slot s -> global chunk for group-rank j."""
    return [15 - j, 8 + j, 7 - j, j]


def _width(t):
    """Prefix width of valid q columns for kblock t (slot budgets 16/12/8/4)."""
    return 512 - 128 * (t // 4)


def _build_module(debug=False, single=False, reps=1):
    import concourse.bacc as bacc
    import concourse.tile as tile
    from concourse import mybir

    F32 = mybir.dt.float32
    BF16 = mybir.dt.bfloat16
    F8 = mybir.dt.float8e4
    AF = mybir.ActivationFunctionType
    Alu = mybir.AluOpType
    DR = mybir.MatmulPerfMode.DoubleRow

    nc = bacc.Bacc("TRN2", target_bir_lowering=False, debug=False,
                   num_devices=1 if single else N_CORES)

    # ---- per-core inputs ----
    xT_d = nc.dram_tensor("xT", [128, 8, TOK], F32, kind="ExternalInput").ap()
    xTh_d = nc.dram_tensor("xTh", [128, 8, TOK], BF16,
                           kind="ExternalInput").ap()
    mask_d = nc.dram_tensor("mask", [128, NKB, 128], BF16,
                            kind="ExternalInput").ap()
    # ---- shared inputs (same data on every core) ----
    wq_d = nc.dram_tensor("wq", [128, 8, D], BF16, kind="ExternalInput").ap()
    wk_d = nc.dram_tensor("wk", [128, 8, D], BF16, kind="ExternalInput").ap()
    wv_d = nc.dram_tensor("wv", [128, 8, D], BF16, kind="ExternalInput").ap()
    wop_d = nc.dram_tensor("wop", [8, 128, 8, 128], BF16,
                           kind="ExternalInput").ap()
    w1_d = nc.dram_tensor("w1", [32, 128, 8, 128], BF16,
                          kind="ExternalInput").ap()
    w2_d = nc.dram_tensor("w2", [8, 128, 32, 128], BF16,
                          kind="ExternalInput").ap()
    # bias cols: 0:8 bq, 8:16 bk, 16:24 bop, 24:32 b2, 32:64 b1
    bias_d = nc.dram_tensor("bias", [128, 64], F32, kind="ExternalInput").ap()
    # sel row 64: cols [hh*128 + p] = 1 if p//64 == hh (head-half select,
    # used as a rank-1 stationary from partition 64)
    sel_d = nc.dram_tensor("sel", [128, 256], BF16, kind="ExternalInput").ap()

    out_d = nc.dram_tensor("outT", [D, TOK], F32, kind="ExternalOutput").ap()
    if debug:
        dbg_qT = nc.dram_tensor("dbg_qT", [D, TOK], F32,
                                kind="ExternalOutput").ap()
        dbg_attnT = nc.dram_tensor("dbg_attnT", [D, TOK], F32,
                                   kind="ExternalOutput").ap()
        dbg_hresT = nc.dram_tensor("dbg_hresT", [D, TOK], F32,
                                   kind="ExternalOutput").ap()

    groups = [[0, 1, 2, 3], [4, 5, 6, 7]]

    with tile.TileContext(nc) as tc:
      for _rep in range(reps):
        with (
            tc.tile_pool(name="const", bufs=1) as constp,
            tc.tile_pool(name="mid", bufs=1) as midp,
            tc.tile_pool(name="w1pre", bufs=1) as w1p,
            tc.tile_pool(name="osb", bufs=3) as osbp,
            tc.tile_pool(name="ps2", bufs=2, space="PSUM") as psm,
            tc.tile_pool(name="ps_pv", bufs=3, space="PSUM") as pspv,
            tc.tile_pool(name="ps_bc", bufs=1, space="PSUM") as psbc,
            tc.tile_pool(name="dram", bufs=1, space="DRAM") as dramp,
        ):
            def pp_tile(name):
                """[128, 512] f32 PSUM accumulator (half of a 2-bank tile)."""
                return psm.tile([128, 2, TOK], F32, name=name,
                                tag="ps2")[:, 0, :]

            # ---------- startup loads (first-use order; big late-use ones
            # stream during attention) ----------
            xTh = constp.tile([128, 8, TOK], BF16, name="xTh")
            nc.sync.dma_start(xTh[:, 0:1, :], xTh_d[:, 0:1, :])
            nc.sync.dma_start(xTh[:, 1:3, :], xTh_d[:, 1:3, :])
            nc.sync.dma_start(xTh[:, 3:8, :], xTh_d[:, 3:8, :])
            bias_t = constp.tile([128, 64], F32, name="bias_t")
            nc.sync.dma_start(bias_t[:], bias_d)
            sel2_t = constp.tile([128, 256], BF16, name="sel2_t")
            nc.sync.dma_start(sel2_t[:], sel_d)
            bq_t = bias_t[:, 0:8]
            bk_t = bias_t[:, 8:16]
            bop_t = bias_t[:, 16:24]
            b2_t = bias_t[:, 24:32]
            b1_t = bias_t[:, 32:64]
            xT = constp.tile([128, 8, TOK], F32, name="xT")
            mask_t = constp.tile([128, NKB, 128], BF16, name="mask_t")

            # cross-phase tiles
            qT = midp.tile([128, 8, TOK], BF16, name="qT")
            attnT = midp.tile([128, 8, TOK], BF16, name="attnT")
            hresT = midp.tile([128, 8, TOK], F32, name="hresT")
            hres8 = midp.tile([128, 8, TOK], BF16, name="hres8")
            wopa = midp.tile([128, 8, 8, 128], BF16, name="wopa")

            # merged KV AllGather buffers, split in two so the deep half
            # (slots 3,2 — attention sub-pass A) gathers early. Per rank
            # half: [2 slots, 128, KVC]: cols 0:1024 = K^T block ([8 dchunk,
            # 128 key-tok], dims on partitions); cols 1024:2064 = V blocks
            # ([8 dchunk, 130]: per dchunk [64 even-head dims | 1 | 64
            # odd-head dims | 1], keys on partitions). kv_a rows = slots
            # (2, 3); kv_b rows = slots (0, 1).
            kv_a = dramp.tile([2, 128, KVC], BF16, name="kv_a")
            kv_b = dramp.tile([2, 128, KVC], BF16, name="kv_b")
            kvg_a = dramp.tile([8, 128, KVC], BF16, name="kvg_a")
            kvg_b = dramp.tile([8, 128, KVC], BF16, name="kvg_b")

            with (
                tc.tile_pool(name="wproj", bufs=3) as wprojp,
                tc.tile_pool(name="workA", bufs=3) as workA,
            ):
                # ---- K^T ----
                wk_t = wprojp.tile([128, 8, D], BF16, name="wk_t", tag="wproj")
                nc.sync.dma_start(wk_t[:, 0:1, :], wk_d[:, 0:1, :])
                nc.sync.dma_start(wk_t[:, 1:3, :], wk_d[:, 1:3, :])
                nc.sync.dma_start(wk_t[:, 3:8, :], wk_d[:, 3:8, :])
                for m in range(8):
                    pp = pp_tile("pp_k")
                    for k in range(8):
                        nc.tensor.matmul(
                            pp, wk_t[:, k, m * 128:(m + 1) * 128],
                            xTh[:, k, :], start=(k == 0), stop=(k == 7))
                    # stage PAIRS of m-blocks slot-major ([128, 4 slots,
                    # 2 m, 128 tok]) so the kv_a/kv_b writes have 256-token
                    # (512B) contiguous runs on both sides — 256B runs pay
                    # a 2x DMA penalty.
                    if m % 2 == 0:
                        kt2 = workA.tile([128, 4, 2, 128], BF16, name="kt2",
                                         tag="work")
                    nc.vector.tensor_scalar_add(
                        kt2[:, :, m % 2, :],
                        pp.rearrange("p (s q) -> p s q", s=4),
                        bk_t[:, m:m + 1])
                    if m % 2 == 1:
                        # tokens are slot-ordered: slots 0,1 -> kv_b;
                        # slots 2,3 -> kv_a
                        nc.sync.dma_start(
                            kv_a[:, :, (m - 1) * 128:(m + 1) * 128]
                            .rearrange("s p c -> p s c"),
                            kt2[:, 2:4, :, :]
                            .rearrange("p s i q -> p s (i q)"))
                        nc.sync.dma_start(
                            kv_b[:, :, (m - 1) * 128:(m + 1) * 128]
                            .rearrange("s p c -> p s c"),
                            kt2[:, 0:2, :, :]
                            .rearrange("p s i q -> p s (i q)"))

                # ---- V (+ones) ----
                wv_t = wprojp.tile([128, 8, D], BF16, name="wv_t", tag="wproj")
                nc.sync.dma_start(wv_t[:], wv_d[:])
                # slots 3, 2 first: they feed the early AllGather (kv_a)
                for s4 in (3, 2, 1, 0):
                    dst = kv_a[s4 - 2] if s4 >= 2 else kv_b[s4]
                    for half in range(2):
                        pp = pp_tile("pp_v")
                        for k in range(8):
                            nc.tensor.matmul(
                                pp, xTh[:, k, s4 * 128:(s4 + 1) * 128],
                                wv_t[:, k, half * 512:(half + 1) * 512],
                                start=(k == 0), stop=(k == 7))
                        v_sb = workA.tile([128, 4, 130], BF16, name="v_sb",
                                          tag="work")
                        nc.vector.tensor_copy(
                            v_sb[:].rearrange("p k (hh c) -> p k hh c", hh=2)
                            [:, :, :, 0:64],
                            pp.rearrange("p (k hh w) -> p k hh w",
                                         k=4, hh=2))
                        nc.vector.memset(
                            v_sb[:].rearrange("p k (hh c) -> p k hh c", hh=2)
                            [:, :, :, 64:65], 1.0)
                        nc.sync.dma_start(
                            dst[:, 1024 + half * 520:1024 + (half + 1) * 520]
                            .rearrange("p (k c) -> p k c", k=4), v_sb[:])
                    if s4 == 2:
                        # deep half complete -> gather it now
                        if single:
                            for r in range(4):
                                nc.sync.dma_start(kvg_a[r * 2:(r + 1) * 2],
                                                  kv_a[:])
                        else:
                            nc.gpsimd.collective_compute(
                                "AllGather", Alu.bypass,
                                replica_groups=groups,
                                ins=[kv_a.opt()], outs=[kvg_a.opt()])

                # ---- AllGather (shallow half) ----
                if single:
                    for r in range(4):
                        nc.sync.dma_start(kvg_b[r * 2:(r + 1) * 2], kv_b[:])
                else:
                    nc.gpsimd.collective_compute(
                        "AllGather", Alu.bypass, replica_groups=groups,
                        ins=[kv_b.opt()], outs=[kvg_b.opt()])

                # mask is needed at the top of attention; small, load now
                nc.sync.dma_start(mask_t[:], mask_d[:])

                # ---- Q^T (host folded 1/8 into wq/bq) ----
                wq_t = wprojp.tile([128, 8, D], BF16, name="wq_t", tag="wproj")
                nc.sync.dma_start(wq_t[:], wq_d[:])
                for m in range(8):
                    pp = pp_tile("pp_q")
                    for k in range(8):
                        nc.tensor.matmul(
                            pp, wq_t[:, k, m * 128:(m + 1) * 128],
                            xTh[:, k, :], start=(k == 0), stop=(k == 7))
                    nc.scalar.activation(qT[:, m, :], pp, AF.Identity,
                                         bias=bq_t[:, m:m + 1])

            if debug:
                nc.sync.dma_start(
                    dbg_qT[:], qT[:].rearrange("p m q -> (m p) q"))

            # ---------- attention ----------
            with (
                tc.tile_pool(name="kv", bufs=1) as kvp,
                tc.tile_pool(name="workB", bufs=3) as workB,
            ):
                # two merged loads per rank: slots {2,3} (deep, attention
                # sub-pass A) then slots {0,1} (sub-pass B) — A's tiles land
                # first so attention starts while B still streams.
                kvra, kvrb = [], []
                for r in range(4):
                    t_a = kvp.tile([128, 2, KVC], BF16, name=f"kvra_{r}",
                                   tag=f"kvra{r}")
                    kvra.append(t_a)
                # slot 3 (kblocks 0..3, processed first) lands before slot 2
                for r in range(4):
                    nc.sync.dma_start(kvra[r][:, 1, :], kvg_a[r * 2 + 1])
                for r in range(4):
                    nc.sync.dma_start(kvra[r][:, 0, :], kvg_a[r * 2])
                for r in range(4):
                    t_b = kvp.tile([128, 2, KVC], BF16, name=f"kvrb_{r}",
                                   tag=f"kvrb{r}")
                    kvrb.append(t_b)

                def kv_slice(t):
                    r, s = _chunk_rank_slot(t)
                    if s >= 2:
                        return kvra[r], s - 2
                    return kvrb[r], s

                def kth(t, h2, m2):
                    tile_, si = kv_slice(t)
                    return tile_[h2 * 64:(h2 + 1) * 64, si,
                                 m2 * 128:(m2 + 1) * 128]

                def vth(t, h2, m2):
                    tile_, si = kv_slice(t)
                    c0 = 1024 + m2 * 130 + h2 * 65
                    return tile_[:, si, c0:c0 + 65]

                # prefetches that stream during attention. Same (sync) queue
                # as the kvr loads: queue FIFO keeps them BEHIND kvr, so they
                # fill the DMA-idle attention window instead of preempting
                # the QKV weight streams at t=0.
                w1a = w1p.tile([128, N_W1_PRE, 8, 128], BF16, name="w1a")
                nc.sync.dma_start(
                    wopa[:], wop_d[:].rearrange("m p k q -> p m k q"))
                nc.sync.dma_start(xT[:], xT_d[:])
                nc.sync.dma_start(
                    w1a[:], w1_d[0:N_W1_PRE].rearrange("n p k q -> p n k q"))
                # kvrb data isn't consumed until sub-pass B — load after the
                # prefetches so it can't head-of-line-block them on the
                # queue while AllGather b is still in flight
                for r in range(4):
                    nc.sync.dma_start(
                        kvrb[r][:],
                        kvg_b[r * 2:r * 2 + 2].rearrange("s p c -> p s c"))

                def do_pair(pi, h, prs):
                    """Score+exp+mask for kblocks (2pi, 2pi+1) — same causal
                    width — sharing one 2-bank PSUM tile so exp and mask
                    each run as a single wide instruction."""
                    m2, h2 = h // 2, h % 2
                    t0 = 2 * pi
                    wt = _width(t0)
                    st2 = psm.tile([128, 2, TOK], F32, name="st2", tag="ps2")
                    for i in range(2):
                        nc.tensor.matmul(
                            st2[:, i, 0:wt], kth(t0 + i, h2, m2),
                            qT[h2 * 64:(h2 + 1) * 64, m2, 0:wt],
                            start=True, stop=True)
                    pr2 = workB.tile([128, 2, TOK], BF16, name="pr2",
                                     tag="pr")
                    nc.scalar.activation(pr2[:, :, 0:wt],
                                         st2[:, :, 0:wt], AF.Exp)
                    nc.vector.tensor_mul(pr2[:, :, wt - 128:wt],
                                         pr2[:, :, wt - 128:wt],
                                         mask_t[:, t0:t0 + 2, :])
                    prs[pi] = pr2

                def pv_pass(h, p0, p1):
                    """Accumulate kblock pairs [p0, p1) into one PSUM bank,
                    score/exp pipelined one pair ahead."""
                    m2, h2 = h // 2, h % 2
                    pv = pspv.tile([65, TOK], F32, name="pv", tag="pv")
                    prs = {}
                    do_pair(p0, h, prs)
                    if p0 + 1 < p1:
                        do_pair(p0 + 1, h, prs)
                    for pi in range(p0, p1):
                        if pi + 2 < p1:
                            do_pair(pi + 2, h, prs)
                        wt = _width(2 * pi)
                        pr2 = prs.pop(pi)
                        for i in range(2):
                            nc.tensor.matmul(
                                pv[:, 0:wt], vth(2 * pi + i, h2, m2),
                                pr2[:, i, 0:wt],
                                start=(pi == p0 and i == 0),
                                stop=(pi == p1 - 1 and i == 1))
                    return pv

                # sub-pass A: deep kblocks 0..7 (widths 512/384, slots 3/2)
                attnP = {}
                for h in range(16):
                    pv = pv_pass(h, 0, 4)
                    attnP[h] = workB.tile([65, TOK], BF16, name=f"aP{h}",
                                          tag=f"aP{h}", bufs=1)
                    with nc.allow_low_precision(reason="bf16 attn partials"):
                        nc.vector.tensor_copy(attnP[h][:], pv[:])

                def do_quad(qi, h, prs):
                    """Score+exp+mask for four same-width kblocks
                    (8+4qi .. 11+4qi): all four score outputs share one
                    2-bank PSUM tile (widths 256/128 fit), so exp runs as a
                    single flat instruction and the mask as one [128,4,128]
                    multiply."""
                    m2, h2 = h // 2, h % 2
                    t0 = 8 + 4 * qi
                    wt = _width(t0)
                    st2 = psm.tile([128, 2, TOK], F32, name="st4", tag="ps2")
                    stf = st2[:].rearrange("p a q -> p (a q)")
                    for i in range(4):
                        nc.tensor.matmul(
                            stf[:, i * wt:(i + 1) * wt], kth(t0 + i, h2, m2),
                            qT[h2 * 64:(h2 + 1) * 64, m2, 0:wt],
                            start=True, stop=True)
                    pr4 = workB.tile([128, 2, TOK], BF16, name="pr4",
                                     tag="pr")
                    prf = pr4[:].rearrange("p a q -> p (a q)")
                    nc.scalar.activation(prf[:, 0:4 * wt], stf[:, 0:4 * wt],
                                         AF.Exp)
                    nc.vector.tensor_mul(
                        prf[:, 0:4 * wt]
                        .rearrange("p (c q) -> p c q", c=4)[:, :, wt - 128:wt],
                        prf[:, 0:4 * wt]
                        .rearrange("p (c q) -> p c q", c=4)[:, :, wt - 128:wt],
                        mask_t[:, t0:t0 + 4, :])
                    prs[qi] = prf

                def pv_pass_b(h):
                    m2, h2 = h // 2, h % 2
                    pv = pspv.tile([65, TOK], F32, name="pv", tag="pv")
                    prs = {}
                    do_quad(0, h, prs)
                    do_quad(1, h, prs)
                    for qi in range(2):
                        wt = _width(8 + 4 * qi)
                        prf = prs.pop(qi)
                        for i in range(4):
                            nc.tensor.matmul(
                                pv[:, 0:wt], vth(8 + 4 * qi + i, h2, m2),
                                prf[:, i * wt:(i + 1) * wt],
                                start=(qi == 0 and i == 0),
                                stop=(qi == 1 and i == 3))
                    return pv

                # sub-pass B: kblocks 8..15 (widths 256/128, slots 1/0),
                # then per-pair softmax normalization. The reciprocal stays
                # on partition 64 (where the ones-row denominator lives) and
                # is broadcast to all 128 partitions by two rank-1 matmuls
                # against a head-half selector row — no cross-partition DMA.
                recip2 = None
                for h in range(16):
                    h2 = h % 2
                    pv = pv_pass_b(h)
                    with nc.allow_low_precision(reason="bf16 attn partials"):
                        nc.vector.tensor_add(attnP[h][:, 0:256],
                                             attnP[h][:, 0:256],
                                             pv[:, 0:256])
                    if h % 2 == 0:
                        recip2 = workB.tile([128, 2, TOK], BF16,
                                            name="recip2", tag="recip2",
                                            bufs=2)
                    with nc.allow_low_precision(reason="bf16 softmax recip"):
                        nc.vector.reciprocal(recip2[64:65, h2, :],
                                             attnP[h][64:65, :])
                    if h % 2 == 1:
                        m = h // 2
                        bc = psbc.tile([128, TOK], F32, name="bc", tag="bc")
                        for hh in range(2):
                            nc.tensor.matmul(
                                bc[:], sel2_t[64:65,
                                              hh * 128:(hh + 1) * 128],
                                recip2[64:65, hh, :],
                                start=(hh == 0), stop=(hh == 1))
                        for hh in range(2):
                            nc.vector.tensor_mul(
                                attnT[hh * 64:(hh + 1) * 64, m, :],
                                attnP[h - 1 + hh][0:64, :],
                                bc[hh * 64:(hh + 1) * 64, :])

            if debug:
                dT = midp.tile([128, 8, TOK], F32, name="dT")
                nc.vector.tensor_copy(dT[:], attnT[:])
                nc.sync.dma_start(
                    dbg_attnT[:], dT[:].rearrange("p m q -> (m p) q"))

            # ---------- output projection (Wo@Wp fused) + residual ----------
            for m in range(8):
                pp = pp_tile("pp_o")
                for k in range(8):
                    nc.tensor.matmul(
                        pp, wopa[:, m, k, :],
                        attnT[:, k, :], start=(k == 0), stop=(k == 7))
                nc.vector.scalar_tensor_tensor(
                    hresT[:, m, :], pp, bop_t[:, m:m + 1], xT[:, m, :],
                    op0=Alu.add, op1=Alu.add)
                nc.scalar.activation(hres8[:, m, :], hresT[:, m, :],
                                     AF.Identity)

            if debug:
                nc.sync.dma_start(
                    dbg_hresT[:], hresT[:].rearrange("p m q -> (m p) q"))

            # ---------- FFN ----------
            with (
                tc.tile_pool(name="gelu", bufs=1) as gelup,
                tc.tile_pool(name="wffn", bufs=3) as wffnp,
            ):
                geluT = gelup.tile([128, 32, TOK], BF16, name="geluT")
                for nf in range(32):
                    if nf < N_W1_PRE:
                        w1s = w1a[:, nf, :, :]
                    else:
                        w1_t = wffnp.tile([128, 8, 128], BF16, name="w1_t",
                                          tag="w1")
                        nc.scalar.dma_start(w1_t[:], w1_d[nf])
                        w1s = w1_t[:]
                    pp = pp_tile("pp_f1")
                    for k in range(8):
                        nc.tensor.matmul(pp, w1s[:, k, :], hres8[:, k, :],
                                         start=(k == 0), stop=(k == 7))
                    nc.scalar.activation(geluT[:, nf, :], pp, AF.Gelu,
                                         bias=b1_t[:, nf:nf + 1])
                for m in range(8):
                    w2_t = wffnp.tile([128, 32, 128], BF16, name="w2_t",
                                      tag="w2", bufs=2)
                    nc.scalar.dma_start(w2_t[:], w2_d[m])
                    pp = pp_tile("pp_f2")
                    for kf in range(32):
                        nc.tensor.matmul(pp, w2_t[:, kf, :],
                                         geluT[:, kf, :], start=(kf == 0),
                                         stop=(kf == 31))
                    out_sb = osbp.tile([128, TOK], F32, name="out_sb",
                                       tag="osb")
                    nc.vector.scalar_tensor_tensor(
                        out_sb[:], pp, b2_t[:, m:m + 1], hresT[:, m, :],
                        op0=Alu.add, op1=Alu.add)
                    nc.sync.dma_start(out_d[m * 128:(m + 1) * 128, :],
                                      out_sb[:])

    nc.compile()
    return nc


def _get_module():
    if "nc" not in _CACHE:
        _CACHE["nc"] = _build_module()
    return _CACHE["nc"]


def _prep_shared(Wq, bq, Wk, bk, Wv, bv, Wo, bo, Wp, bp, W1, b1, W2, b2):
    """Host-side weight preprocessing (fp32 in, blocked bf16/f32 out)."""
    from concourse import mybir
    BF = mybir.dt.np(mybir.dt.bfloat16)
    F8 = mybir.dt.np(mybir.dt.float8e4)
    Wq_s = (Wq.astype(np.float64) * 0.125).astype(np.float32)
    bq_s = (bq.astype(np.float64) * 0.125).astype(np.float32)
    Wop = (Wo.astype(np.float64) @ Wp.astype(np.float64)).astype(np.float32)
    bop = (bv.astype(np.float64) @ Wo.astype(np.float64) @ Wp.astype(np.float64)
           + bo.astype(np.float64) @ Wp.astype(np.float64)
           + bp.astype(np.float64)).astype(np.float32)
    bias = np.zeros((128, 64), np.float32)
    bias[:, 0:8] = bq_s.reshape(8, 128).T
    bias[:, 8:16] = bk.reshape(8, 128).T
    bias[:, 16:24] = bop.reshape(8, 128).T
    bias[:, 24:32] = b2.reshape(8, 128).T
    bias[:, 32:64] = b1.reshape(32, 128).T
    return {
        "wq": np.ascontiguousarray(
            Wq_s.reshape(8, 128, D).transpose(1, 0, 2)).astype(BF),
        "wk": np.ascontiguousarray(
            Wk.reshape(8, 128, D).transpose(1, 0, 2)).astype(BF),
        "wv": np.ascontiguousarray(
            Wv.reshape(8, 128, D).transpose(1, 0, 2)).astype(BF),
        "wop": np.ascontiguousarray(
            Wop.reshape(8, 128, 8, 128).transpose(2, 1, 0, 3)).astype(BF),
        "w1": np.ascontiguousarray(
            W1.reshape(8, 128, 32, 128).transpose(2, 1, 0, 3)).astype(BF),
        "w2": np.ascontiguousarray(
            W2.reshape(32, 128, 8, 128).transpose(2, 1, 0, 3)).astype(BF),
        "bias": bias,
        "sel": _sel64(),
    }


def _sel64():
    """[128, 256] head-half selector: row 64, cols hh*128+p = (p//64 == hh)."""
    from concourse import mybir
    BF = mybir.dt.np(mybir.dt.bfloat16)
    sel = np.zeros((128, 256), np.float32)
    p = np.arange(128)
    sel[64, 0:128] = (p // 64 == 0)
    sel[64, 128:256] = (p // 64 == 1)
    return sel.astype(BF)


def _prep_core(x, core):
    """Per-core inputs: xT (feature-major, slot order, f32+bf16) and mask."""
    from concourse import mybir
    BF = mybir.dt.np(mybir.dt.bfloat16)
    b, j = core // 4, core % 4
    chunks = _rank_chunks(j)
    xc = np.concatenate(
        [x[b, c * 128:(c + 1) * 128, :] for c in chunks], axis=0)  # [512, D]
    xT = np.ascontiguousarray(
        xc.T.reshape(8, 128, TOK).transpose(1, 0, 2))  # [128, 8, TOK]
    mask = np.zeros((NKB, 128, 128), np.float32)
    ki = np.arange(128)[:, None]
    qi = np.arange(128)[None, :]
    for t in range(NKB):
        s = 3 - t // 4
        c = chunks[s]
        mask[t] = ((c * 128 + qi) >= (t * 128 + ki)).astype(np.float32)
    return {"xT": xT, "xTh": xT.astype(BF),
            "mask": np.ascontiguousarray(mask.transpose(1, 0, 2)).astype(BF)}


def kernel(x, Wq, bq, Wk, bk, Wv, bv, Wo, bo, Wp, bp, W1, b1, W2, b2):
    from concourse.bass_utils import run_bass_kernel_spmd

    x = np.asarray(x, np.float32)
    shared = _prep_shared(np.asarray(Wq), np.asarray(bq), np.asarray(Wk),
                          np.asarray(bk), np.asarray(Wv), np.asarray(bv),
                          np.asarray(Wo), np.asarray(bo), np.asarray(Wp),
                          np.asarray(bp), np.asarray(W1), np.asarray(b1),
                          np.asarray(W2), np.asarray(b2))
    in_maps = []
    for c in range(N_CORES):
        m = dict(shared)
        m.update(_prep_core(x, c))
        in_maps.append(m)

    nc = _get_module()
    res = run_bass_kernel_spmd(nc, in_maps, core_ids=list(range(N_CORES)))
    _CACHE["last_results"] = res

    out = np.empty((B, S, D), np.float32)
    for c in range(N_CORES):
        b, j = c // 4, c % 4
        chunks = _rank_chunks(j)
        outT = res.results[c]["outT"]  # [D, 512]
        for s, ch in enumerate(chunks):
            out[b, ch * 128:(ch + 1) * 128, :] = \
                outT[:, s * 128:(s + 1) * 128].T
    return out
